# revision 10
# baseline (speedup 1.0000x reference)
"""Masked attention-aggregator kernel for Trainium2 (8 NeuronCores, SPMD).

Reference computation (B=16, N=2048, D=128, DQ=64), all fp32:
    q = x @ Wq.T + bq                      [B, N, DQ]
    k = x @ Wk.T + bk                      [B, N, DQ]
    s = (k @ q.T) / sqrt(DQ)               [B, N, N]   (s[b,n,m] = k[n].q[m])
    w = softmax(s + (mask[m]>0 ? 0 : -1e9), axis=m)
    out = w @ x                            [B, N, D]

Strategy: data-parallel over batch (2 batches per core).  Per batch, a
flash-style streaming attention that never materializes [N, N] anywhere.

Key transfer trick: the n axis is PERMUTED on the host so the mask-kept
columns come first.  The computation is n-equivariant (keys k[n] and
output rows out[n] just follow the permutation; the host un-permutes the
output).  The aggregated (m) axis is then simply the first mcp permuted
columns, so ONE copy of x per core serves as:
  * keys-projection source      xt[:, 0:N]
  * queries-projection source   xt[:, 0:mcp]       (same buffer, sliced)
  * numerator source            xbar-DMA-transposed [128,128] chunks of
                                xt[:, 0:mcp] (m on partitions)
Columns cnt..mcp hold *dropped* (masked-out) x values, not zeros; they
are killed by a -30000 additive penalty riding a 65th contraction row of
the scores matmul (qt row 64 = penalty row, kt row 64 = all-ones), which
drives their softmax weight to exactly 0 in both numerator and
denominator.

All tensor data travels as float16 (host-quantized): halves transfer and
SBUF/DMA traffic, doubles DVE throughput.  PSUM accumulation stays fp32;
measured end-to-end error ~5e-4 vs the 2e-2 gate.  The 1/sqrt(DQ) scale
is folded into Wq on the host.

Scores are computed transposed, ST[m, n] = q_s[m].k[n], with m on PSUM
partitions, so E^T = exp(ST) lands in SBUF (f16) already laid out as the
rhs for the numerator (xcb_chunk^T @ E^T -> out^T[d, n], PSUM fp32).
The denominator accumulates E^T chunk-wise on the DVE in f16 (2x DVE
mode; f16's 10-bit mantissa keeps the den error ~1e-3), then one ones^T
matmul per group reduces across partitions and replicates den to all
128 partitions (fp32 PSUM), so the final divide is a plain elementwise
op (reciprocal_approx_fast + multiply on the DVE).

Output is produced transposed ([D, N] per batch, f16, one store per
batch) and un-permuted / un-transposed / upcast on the host.
"""

import math
import os

import numpy as np

B, N, D, DQ = 16, 2048, 128, 64
NCORES = 8
BPC = B // NCORES  # batches per core

_cache = {}


def _build_program(mcp: int, reps: int = 1, NG: int = 1024):
    """Build the per-core Bass program for a compacted/padded m-size of mcp."""
    import concourse.bass as bass
    import concourse.tile as tile
    from concourse import bacc, mybir

    f32 = mybir.dt.float32
    f32r = mybir.dt.float32r
    f16 = mybir.dt.float16
    mc = mcp // 128  # number of m chunks
    ngroups = N // NG

    nc = bacc.Bacc("TRN2", target_bir_lowering=False, debug=False, num_devices=1)

    xt = nc.dram_tensor("xt", [BPC, D, N], f16, kind="ExternalInput").ap()
    pen = nc.dram_tensor("pen", [BPC, 1, mcp], f16, kind="ExternalInput").ap()
    onerow = nc.dram_tensor("onerow", [1, N], f16, kind="ExternalInput").ap()
    auxw = nc.dram_tensor("auxw", [D, 2 * DQ], f16, kind="ExternalInput").ap()
    bias = nc.dram_tensor("bias", [DQ, 2], f32, kind="ExternalInput").ap()
    out = nc.dram_tensor("out", [BPC, D, N], f16, kind="ExternalOutput").ap()

    with tile.TileContext(nc) as tc:
        with (
            tc.tile_pool(name="singles", bufs=1) as singles,
            tc.tile_pool(name="xtp", bufs=2) as xtp,
            tc.tile_pool(name="xcp", bufs=2) as xcp,
            tc.tile_pool(name="qtp", bufs=2) as qtp,
            tc.tile_pool(name="ktp", bufs=2) as ktp,
            tc.tile_pool(name="etp", bufs=10) as etp,
            tc.tile_pool(name="eap", bufs=2) as eap,
            tc.tile_pool(name="rdp", bufs=2) as rdp,
            tc.tile_pool(name="nrmp", bufs=2) as nrmp,
            tc.tile_pool(name="st", bufs=2, space="PSUM") as stp,
            tc.tile_pool(name="oa", bufs=2, space="PSUM") as oap,
        ):
            auxw_sb = singles.tile([D, 2 * DQ], f16)
            nc.sync.dma_start(auxw_sb[:], auxw[:])
            bias_sb = singles.tile([DQ, 2], f32)
            nc.sync.dma_start(bias_sb[:], bias[:])
            ones = singles.tile([128, 128], f16)
            nc.vector.memset(ones[:], 1.0)

            def body():
              for b in range(BPC):
                # ---- load (one copy of x per batch) ----
                xt_t = xtp.tile([D, N], f16, tag="xt")
                nc.sync.dma_start(xt_t[:], xt[b])

                # m-chunk-major compacted x via xbar DMA transpose
                xcb_t = xcp.tile([128, mc * D], f16, tag="xc")
                nc.sync.dma_start_transpose(
                    xcb_t[:].rearrange("p (m d) -> p m d", m=mc),
                    xt_t[:, 0:mcp])

                # ---- projections (into [dq+1, m] / [dq+1, n] layout) ----
                def project(dst, w_col, src_w, b_col):
                    for j0 in range(0, src_w, NG):
                        span = min(NG, src_w - j0)
                        pp = stp.tile([128, NG], f32, tag="st")
                        for j in range(0, span, 512):
                            jw = min(512, span - j)
                            nc.tensor.matmul(
                                pp[0:DQ, j:j + jw],
                                auxw_sb[:, w_col * DQ:(w_col + 1) * DQ],
                                xt_t[:, j0 + j:j0 + j + jw],
                                start=True, stop=True)
                        nc.vector.tensor_scalar_add(
                            dst[0:DQ, j0:j0 + span], pp[0:DQ, 0:span],
                            bias_sb[:, b_col:b_col + 1])

                qt_t = qtp.tile([DQ + 1, mcp], f16, tag="qt")
                nc.gpsimd.dma_start(qt_t[DQ:DQ + 1, :], pen[b])
                project(qt_t, 0, mcp, 0)
                kt_t = ktp.tile([DQ + 1, N], f16, tag="kt")
                nc.gpsimd.dma_start(kt_t[DQ:DQ + 1, :], onerow[:])
                project(kt_t, 1, N, 1)

                # ---- attention over n-groups ----
                nrm = nrmp.tile([128, N], f16, tag="nrm")
                for g in range(ngroups):
                    oa = oap.tile([128, NG], f32, tag="oa")
                    eacc = eap.tile([128, NG], f16, tag="eacc")
                    for m in range(mc):
                        st = stp.tile([128, NG], f32, tag="st")
                        for h in range(NG // 512):
                            nc.tensor.matmul(
                                st[:, h * 512:(h + 1) * 512],
                                qt_t[:, m * 128:(m + 1) * 128],
                                kt_t[:, g * NG + h * 512: g * NG + (h + 1) * 512],
                                start=True, stop=True)
                        et = etp.tile([128, NG], f16, tag="et")
                        nc.scalar.activation(et[:], st[:],
                                             mybir.ActivationFunctionType.Exp)
                        first, last = (m == 0), (m == mc - 1)
                        for h in range(NG // 512):
                            hs = slice(h * 512, (h + 1) * 512)
                            nc.tensor.matmul(oa[:, hs],
                                             xcb_t[:, m * D:(m + 1) * D],
                                             et[:, hs], start=first, stop=last)
                        if first:
                            nc.vector.tensor_copy(eacc[:], et[:])
                        else:
                            nc.vector.tensor_add(eacc[:], eacc[:], et[:])
                    # den replicated over partitions via ones.T @ eacc
                    dn = stp.tile([128, NG], f32, tag="st")
                    for h in range(NG // 512):
                        hs = slice(h * 512, (h + 1) * 512)
                        nc.tensor.matmul(dn[:, hs], ones[:], eacc[:, hs],
                                         start=True, stop=True)
                    rden = rdp.tile([128, NG], f32, tag="rden")
                    nc.vector.reciprocal_approx_fast(rden[:], dn[:])
                    nc.vector.tensor_mul(nrm[:, g * NG:(g + 1) * NG], oa[:],
                                         rden[:])
                nc.gpsimd.dma_start(out[b][:], nrm[:])

            if reps > 1:
                with tc.For_i(0, reps, 1):
                    body()
            else:
                body()

    nc.compile()
    return nc


def _prep(x, mask, Wq, bq, Wk, bk):
    """Host-side prep: n-permutation (kept cols first), f16 casts, sharding."""
    x = np.asarray(x, dtype=np.float32)
    mask = np.asarray(mask)
    Wq = np.asarray(Wq, dtype=np.float32)
    bq = np.asarray(bq, dtype=np.float32)
    Wk = np.asarray(Wk, dtype=np.float32)
    bk = np.asarray(bk, dtype=np.float32)

    scale = np.float32(1.0 / math.sqrt(DQ))

    perm = np.empty((B, N), dtype=np.int64)
    counts = []
    for b in range(B):
        keep = np.nonzero(mask[b] > 0)[0]
        drop = np.nonzero(mask[b] <= 0)[0]
        perm[b, :len(keep)] = keep
        perm[b, len(keep):] = drop
        counts.append(len(keep))
    mcap = max(max(counts), 1)
    mcp = ((mcap + 127) // 128) * 128

    # x with columns permuted (kept first), transposed to [D, N], f16
    xp = np.take_along_axis(x, perm[:, :, None], axis=1)       # [B, N, D]
    xt = np.ascontiguousarray(xp.transpose(0, 2, 1)).astype(np.float16)

    pen = np.full((B, 1, mcp), -30000.0, dtype=np.float16)
    for b in range(B):
        pen[b, 0, :counts[b]] = 0.0

    auxw = np.concatenate([(Wq * scale).T, Wk.T], axis=1).astype(np.float16)
    bias = np.stack([bq * scale, bk], axis=1).astype(np.float32)  # [DQ, 2]
    one_row = np.ones((1, N), dtype=np.float16)

    in_maps = []
    for c in range(NCORES):
        s = slice(c * BPC, (c + 1) * BPC)
        in_maps.append({
            "xt": xt[s], "pen": pen[s],
            "auxw": auxw, "bias": bias, "onerow": one_row,
        })
    return in_maps, mcp, perm


def kernel(x, mask, Wq, bq, Wk, bk):
    from concourse import bass_utils

    in_maps, mcp, perm = _prep(x, mask, Wq, bq, Wk, bk)

    if mcp not in _cache:
        _cache[mcp] = _build_program(mcp)
    nc = _cache[mcp]

    res = bass_utils.run_bass_kernel_spmd(
        nc, in_maps, core_ids=list(range(NCORES)),
        trace=bool(os.environ.get("BASS_TRACE")),
    )
    kernel._last_results = res

    out_t = np.concatenate([res.results[c]["out"] for c in range(NCORES)], axis=0)
    outp = out_t.astype(np.float32).transpose(0, 2, 1)  # [B, N, D], permuted n
    out = np.empty_like(outp)
    bidx = np.arange(B)[:, None]
    out[bidx, perm] = outp  # inverse permutation of the n axis
    return np.ascontiguousarray(out)
